# revision 10
# baseline (speedup 1.0000x reference)
"""Trainium2 Bass kernel for BroadcastObstaclesToLanes (embedding lookup).

out[m, :] = obs_pos[same_obs_mask[m, 0], :]   m in [0, 16777216)

Sharding: M (lanes) split across 8 NeuronCores; the obs_pos table is
replicated so every core's gather is fully local.

Strategy (two-phase gather):
  Phase 1 — block gather: the table is viewed as 32768 blocks of 32 rows
  (256 B).  For each lane, one dma_gather descriptor fetches the 256 B block
  containing its row (block id = idx >> 5, an int16) into SBUF.  One
  dma_gather instruction moves 8192 blocks (the SWDGE packet ceiling; more
  crashes the Q7 ucode).  Descriptor generation is the scarce resource
  (~7 ns/idx on one Q7 core pair), so the four sub-gathers of each chunk go
  to the four SWDGE queues — each queue runs on its own GPSIMD core pair,
  so generation proceeds 4-wide.

  Phase 2 — on-chip select: each lane picks pair (idx & 31) out of its
  32-pair block on the vector engine in two mux levels (8-way group select
  with an 8-float-wide predicated copy, then 4-way pair select), using
  0-stride broadcast APs for the masks.  ~82 DVE cycles/lane/partition vs
  ~96 for a flat 32-way select, and 5x fewer instructions.

Lane layout per core (2,097,152 lanes = 64 chunks x 4 sub-gathers x 8192):
lane m = c*32768 + q*8192 + r*128 + p lives at SBUF position
[p, q*64 + r]; its int16 block id is read from index partition p%16
(replicated x8 so every GPSIMD core sees a copy).  All host-side reshapes
are pure index layout.
"""

import os

import numpy as np

N_OBS = 1048576
M_LANES = 16777216
NCORES = 8
MS = M_LANES // NCORES  # 2,097,152 lanes per core
P = 128
GNI = 8192  # lanes per dma_gather instruction (SWDGE packet ceiling)
NI = int(os.environ.get("K_NI", "16384"))  # lanes per select chunk
NQ = NI // GNI  # 4 sub-gathers per chunk
GL = GNI // P  # 64 block rows per partition per sub-gather
L = NI // P  # 256 lanes per partition per chunk
L16 = NI // 16  # 2048 index columns in the 16-partition wrap
GL16 = GNI // 16  # 512 index columns per sub-gather
NCH = MS // NI  # 64 chunks per core
EB = 32  # table rows per gathered block (256 B)
ES = 64  # elem_size in f32 (EB rows x 2)
NB = N_OBS // EB  # 32768 blocks

assert MS % NI == 0 and NI % GNI == 0

# tuning knobs (defaults are the production config; env overrides for benching)
BBUFS = int(os.environ.get("K_BBUFS", "5"))
NQUEUES = int(os.environ.get("K_NQUEUES", "4"))
KREPS = int(os.environ.get("K_REPS", "1"))

_cached_nc = None


def _build():
    global _cached_nc
    if _cached_nc is not None:
        return _cached_nc

    import concourse.bacc as bacc
    import concourse.tile as tile
    from concourse import mybir
    from concourse.bass import AP

    nc = bacc.Bacc(
        "TRN2",
        target_bir_lowering=False,
        debug=False,
        num_devices=NCORES,
        num_swdge_queues=NQUEUES,
    )
    tblv = nc.dram_tensor(
        "tblv", [NB, ES], mybir.dt.float32, kind="ExternalInput"
    ).ap()
    hid = nc.dram_tensor(
        "hi", [NCH, P, L16], mybir.dt.int16, kind="ExternalInput"
    ).ap()
    lod = nc.dram_tensor(
        "lo", [NCH, 2, P, L], mybir.dt.float32, kind="ExternalInput"
    ).ap()
    out = nc.dram_tensor(
        "out", [NCH, P, L, 2], mybir.dt.float32, kind="ExternalOutput"
    ).ap()

    def bcast(ap, n):
        # append a 0-stride dim: mask[p, l] -> mask[p, l, n]
        return AP(ap.tensor, ap.offset, list(ap.ap) + [[0, n]])

    with tile.TileContext(nc) as tc:
        with tc.tile_pool(name="bp", bufs=BBUFS) as bp, tc.tile_pool(
            name="ip", bufs=2
        ) as ip, tc.tile_pool(name="lp", bufs=2) as lp, tc.tile_pool(
            name="ap", bufs=2
        ) as ap_:
            g = 0  # global sub-gather counter: rotates queues across chunks
            for c in [cc for _ in range(KREPS) for cc in range(NCH)]:
                hi_t = ip.tile([P, L16], mybir.dt.int16, tag="hi")
                lohi_t = lp.tile([P, L], mybir.dt.float32, tag="lohi")
                lolo_t = lp.tile([P, L], mybir.dt.float32, tag="lolo")
                nc.sync.dma_start(hi_t[:], hid[c])
                nc.sync.dma_start(lohi_t[:], lod[c, 0])
                nc.sync.dma_start(lolo_t[:], lod[c, 1])
                b_t = bp.tile([P, L, ES], mybir.dt.float32, tag="blk")
                for q in range(NQ):
                    nc.gpsimd.dma_gather(
                        out_ap=b_t[:, q * GL : (q + 1) * GL, :],
                        in_ap=tblv[:],
                        idxs_ap=hi_t[:, q * GL16 : (q + 1) * GL16],
                        num_idxs=GNI,
                        num_idxs_reg=GNI,
                        elem_size=ES,
                        single_packet=False,
                        queue_num=g % NQUEUES,
                    )
                    g += 1
                acc8 = ap_.tile([P, L, 8], mybir.dt.float32, tag="acc8")
                acc2 = ap_.tile([P, L, 2], mybir.dt.float32, tag="acc2")
                mask = ap_.tile([P, L], mybir.dt.int8, tag="mask")
                # level 1: select the lane's 8-float group (lo >> 2)
                nc.vector.tensor_copy(acc8[:], b_t[:, :, 0:8])
                for j in range(1, 8):
                    nc.vector.tensor_scalar(
                        mask[:], lohi_t[:], float(j), None,
                        mybir.AluOpType.is_equal,
                    )
                    nc.vector.copy_predicated(
                        acc8[:], bcast(mask[:], 8),
                        b_t[:, :, 8 * j : 8 * j + 8],
                    )
                # level 2: select the pair within the group (lo & 3)
                nc.vector.tensor_copy(acc2[:], acc8[:, :, 0:2])
                for k in range(1, 4):
                    nc.vector.tensor_scalar(
                        mask[:], lolo_t[:], float(k), None,
                        mybir.AluOpType.is_equal,
                    )
                    nc.vector.copy_predicated(
                        acc2[:], bcast(mask[:], 2),
                        acc8[:, :, 2 * k : 2 * k + 2],
                    )
                nc.scalar.dma_start(out[c], acc2[:])

    nc.compile()
    _cached_nc = nc
    return nc


def make_in_maps(obs_pos, same_obs_mask):
    """Host-side index/layout marshalling (pure layout, no value compute)."""
    tblv = np.ascontiguousarray(
        np.asarray(obs_pos, dtype=np.float32)
    ).reshape(NB, ES)
    idx32 = np.asarray(same_obs_mask).reshape(-1).astype(np.int32)

    in_maps = []
    for c in range(NCORES):
        idx = idx32[c * MS : (c + 1) * MS]
        hi = (idx >> 5).astype(np.int16)
        lohi = ((idx >> 2) & 7).astype(np.float32)
        lolo = (idx & 3).astype(np.float32)
        # index n of sub-gather (ch, q) sits at [ch, n%16, q*512 + n//16]
        hi_t = hi.reshape(NCH, NQ, GL16, 16).transpose(0, 3, 1, 2)
        hi_t = np.ascontiguousarray(
            np.broadcast_to(
                hi_t.reshape(NCH, 1, 16, L16), (NCH, 8, 16, L16)
            )
        ).reshape(NCH, P, L16)
        # lane n of sub-gather (ch, q) sits at [ch, n%128, q*64 + n//128]
        lo_t = np.stack(
            [
                v.reshape(NCH, NQ, GL, P).transpose(0, 3, 1, 2).reshape(NCH, P, L)
                for v in (lohi, lolo)
            ],
            axis=1,
        )
        in_maps.append(
            {"tblv": tblv, "hi": hi_t, "lo": np.ascontiguousarray(lo_t)}
        )
    return in_maps


def kernel(obs_pos, same_obs_mask):
    from concourse.bass_utils import run_bass_kernel_spmd

    nc = _build()
    in_maps = make_in_maps(obs_pos, same_obs_mask)
    res = run_bass_kernel_spmd(nc, in_maps, core_ids=list(range(NCORES)))
    outs = []
    for r in res.results:
        o = r["out"]  # [NCH, P, L, 2]; lane c*32768+q*8192+r*128+p at [c,p,q*64+r]
        o = o.reshape(NCH, P, NQ, GL, 2).transpose(0, 2, 3, 1, 4)
        outs.append(o.reshape(MS, 2))
    return np.ascontiguousarray(np.concatenate(outs, axis=0))


# revision 12
# speedup vs baseline: 1.3373x; 1.3373x over previous
"""Trainium2 Bass kernel for BroadcastObstaclesToLanes (embedding lookup).

out[m, :] = obs_pos[same_obs_mask[m, 0], :]   m in [0, 16777216)

Sharding: M (lanes) split across 8 NeuronCores; the obs_pos table is
replicated so every core's gather is fully local.

Strategy (two-phase gather):
  Phase 1 — block gather: the table is viewed as 32768 blocks of 32 rows
  (256 B).  For each lane, one dma_gather descriptor fetches the 256 B block
  containing its row (block id = idx >> 5, an int16) into SBUF.  One
  dma_gather instruction moves 8192 blocks (the SWDGE packet ceiling; more
  crashes the Q7 ucode).  Descriptor generation is the scarce resource
  (~7 ns/idx on one Q7 core pair), so the four sub-gathers of each chunk go
  to the four SWDGE queues — each queue runs on its own GPSIMD core pair,
  so generation proceeds 4-wide.

  Phase 2 — on-chip select: each lane picks pair (idx & 31) out of its
  32-pair block on the vector engine in two mux levels (8-way group select
  with an 8-float-wide predicated copy, then 4-way pair select), using
  0-stride broadcast APs for the masks.  ~82 DVE cycles/lane/partition vs
  ~96 for a flat 32-way select, and 5x fewer instructions.

Lane layout per core (2,097,152 lanes = 64 chunks x 4 sub-gathers x 8192):
lane m = c*32768 + q*8192 + r*128 + p lives at SBUF position
[p, q*64 + r]; its int16 block id is read from index partition p%16
(replicated x8 so every GPSIMD core sees a copy).  All host-side reshapes
are pure index layout.
"""

import os

import numpy as np

N_OBS = 1048576
M_LANES = 16777216
NCORES = 8
MS = M_LANES // NCORES  # 2,097,152 lanes per core
P = 128
GNI = 8192  # lanes per dma_gather instruction (SWDGE packet ceiling)
NI = int(os.environ.get("K_NI", "32768"))  # lanes per select chunk
NQ = NI // GNI  # 4 sub-gathers per chunk
GL = GNI // P  # 64 block rows per partition per sub-gather
L = NI // P  # 256 lanes per partition per chunk
L16 = NI // 16  # 2048 index columns in the 16-partition wrap
GL16 = GNI // 16  # 512 index columns per sub-gather
NCH = MS // NI  # 64 chunks per core
EB = 32  # table rows per gathered block (256 B)
ES = 64  # elem_size in f32 (EB rows x 2)
NB = N_OBS // EB  # 32768 blocks

assert MS % NI == 0 and NI % GNI == 0

# tuning knobs (defaults are the production config; env overrides for benching)
BBUFS = int(os.environ.get("K_BBUFS", "3"))
NQUEUES = int(os.environ.get("K_NQUEUES", "4"))
KREPS = int(os.environ.get("K_REPS", "1"))

_cached_nc = None


def _build():
    global _cached_nc
    if _cached_nc is not None:
        return _cached_nc

    import concourse.bacc as bacc
    import concourse.tile as tile
    from concourse import mybir
    from concourse.bass import AP

    nc = bacc.Bacc(
        "TRN2",
        target_bir_lowering=False,
        debug=False,
        num_devices=NCORES,
        num_swdge_queues=NQUEUES,
    )
    tblv = nc.dram_tensor(
        "tblv", [NB, ES], mybir.dt.float32, kind="ExternalInput"
    ).ap()
    hid = nc.dram_tensor(
        "hi", [NCH, P, L16], mybir.dt.int16, kind="ExternalInput"
    ).ap()
    lod = nc.dram_tensor(
        "lo", [NCH, 2, P, L], mybir.dt.int8, kind="ExternalInput"
    ).ap()
    out = nc.dram_tensor(
        "out", [NCH, P, L, 2], mybir.dt.float32, kind="ExternalOutput"
    ).ap()

    def bcast(ap, n):
        # append a 0-stride dim: mask[p, l] -> mask[p, l, n]
        return AP(ap.tensor, ap.offset, list(ap.ap) + [[0, n]])

    with tile.TileContext(nc) as tc:
        with tc.tile_pool(name="bp", bufs=BBUFS) as bp, tc.tile_pool(
            name="ip", bufs=2
        ) as ip, tc.tile_pool(name="lp", bufs=2) as lp, tc.tile_pool(
            name="ap", bufs=1
        ) as ap_, tc.tile_pool(name="op", bufs=1) as op_:
            g = 0  # global sub-gather counter: rotates queues across chunks
            for c in [cc for _ in range(KREPS) for cc in range(NCH)]:
                hi_t = ip.tile([P, L16], mybir.dt.int16, tag="hi")
                lohi_t = lp.tile([P, L], mybir.dt.int8, tag="lohi")
                lolo_t = lp.tile([P, L], mybir.dt.int8, tag="lolo")
                nc.sync.dma_start(hi_t[:], hid[c])
                nc.sync.dma_start(lohi_t[:], lod[c, 0])
                nc.sync.dma_start(lolo_t[:], lod[c, 1])
                b_t = bp.tile([P, L, ES], mybir.dt.float32, tag="blk")
                for q in range(NQ):
                    nc.gpsimd.dma_gather(
                        out_ap=b_t[:, q * GL : (q + 1) * GL, :],
                        in_ap=tblv[:],
                        idxs_ap=hi_t[:, q * GL16 : (q + 1) * GL16],
                        num_idxs=GNI,
                        num_idxs_reg=GNI,
                        elem_size=ES,
                        single_packet=False,
                        queue_num=g % NQUEUES,
                    )
                    g += 1
                acc4 = ap_.tile([P, L, 4], mybir.dt.float32, tag="acc4")
                mask = ap_.tile([P, L], mybir.dt.int8, tag="mask")
                acc2 = op_.tile([P, L, 2], mybir.dt.float32, tag="acc2")
                # level 1: select the lane's 4-float group (lo >> 1)
                nc.vector.tensor_copy(acc4[:], b_t[:, :, 0:4])
                for j in range(1, 16):
                    nc.vector.tensor_scalar(
                        mask[:], lohi_t[:], j, None,
                        mybir.AluOpType.is_equal,
                    )
                    nc.vector.copy_predicated(
                        acc4[:], bcast(mask[:], 4),
                        b_t[:, :, 4 * j : 4 * j + 4],
                    )
                # level 2: select the pair within the group (lo & 1)
                nc.vector.tensor_copy(acc2[:], acc4[:, :, 0:2])
                nc.vector.tensor_scalar(
                    mask[:], lolo_t[:], 1, None, mybir.AluOpType.is_equal
                )
                nc.vector.copy_predicated(
                    acc2[:], bcast(mask[:], 2), acc4[:, :, 2:4]
                )
                nc.scalar.dma_start(out[c], acc2[:])

    nc.compile()
    _cached_nc = nc
    return nc


def make_in_maps(obs_pos, same_obs_mask):
    """Host-side index/layout marshalling (pure layout, no value compute)."""
    tblv = np.ascontiguousarray(
        np.asarray(obs_pos, dtype=np.float32)
    ).reshape(NB, ES)
    idx32 = np.asarray(same_obs_mask).reshape(-1).astype(np.int32)

    in_maps = []
    for c in range(NCORES):
        idx = idx32[c * MS : (c + 1) * MS]
        hi = (idx >> 5).astype(np.int16)
        lohi = ((idx >> 1) & 15).astype(np.int8)
        lolo = (idx & 1).astype(np.int8)
        # index n of sub-gather (ch, q) sits at [ch, n%16, q*512 + n//16]
        hi_t = hi.reshape(NCH, NQ, GL16, 16).transpose(0, 3, 1, 2)
        hi_t = np.ascontiguousarray(
            np.broadcast_to(
                hi_t.reshape(NCH, 1, 16, L16), (NCH, 8, 16, L16)
            )
        ).reshape(NCH, P, L16)
        # lane n of sub-gather (ch, q) sits at [ch, n%128, q*64 + n//128]
        lo_t = np.stack(
            [
                v.reshape(NCH, NQ, GL, P).transpose(0, 3, 1, 2).reshape(NCH, P, L)
                for v in (lohi, lolo)
            ],
            axis=1,
        )
        in_maps.append(
            {"tblv": tblv, "hi": hi_t, "lo": np.ascontiguousarray(lo_t)}
        )
    return in_maps


def kernel(obs_pos, same_obs_mask):
    from concourse.bass_utils import run_bass_kernel_spmd

    nc = _build()
    in_maps = make_in_maps(obs_pos, same_obs_mask)
    res = run_bass_kernel_spmd(nc, in_maps, core_ids=list(range(NCORES)))
    outs = []
    for r in res.results:
        o = r["out"]  # [NCH, P, L, 2]; lane c*32768+q*8192+r*128+p at [c,p,q*64+r]
        o = o.reshape(NCH, P, NQ, GL, 2).transpose(0, 2, 3, 1, 4)
        outs.append(o.reshape(MS, 2))
    return np.ascontiguousarray(np.concatenate(outs, axis=0))
